# revision 10
# baseline (speedup 1.0000x reference)
"""Multi-head attention (B=2, S=2048, D=1024, H=16) on 8 TRN2 NeuronCores.

Sharding: core c handles batch b = c//4 and head-group hg = c%4 (4 heads).
Device computes, per core:
  Q^T,K^T = (Wq_hg^T q^T + bq), (Wk_hg^T k^T + bk)    [dk-major layouts]
  V4      = v @ Wv_hg + bv, rows masked by key-padding mask, plus a ones
            column per head (yields softmax denominators Z for free)
  scoresT = K @ Q^T per head (scaled via Wq pre-scaling), exp on ScalarE,
  outT    = V_ext^T @ exp  (row 64 = Z), attn = exp * mask * (1/Z)
  partial = (outT/Z heads combined) @ Wo_hg  (+ bo on hg==0 cores)
Host: pre-transposes q/k/v per batch, slices weights per head-group, sums the
4 row-parallel partials per batch, and reassembles attn (device writes attn
transposed [k,q]; the return is a zero-copy transposed view).

All matmuls run in float32r (TF32-like, ~1e-4 rel err, full PE rate).
"""

import sys

sys.path.insert(0, "/opt/trn_rl_repo")

import numpy as np

import concourse.bass as bass
import concourse.mybir as mybir
import concourse.tile as tile

F32 = mybir.dt.float32
F32R = mybir.dt.float32r
AF = mybir.ActivationFunctionType
ALU = mybir.AluOpType

B, S, D, H = 2, 2048, 1024, 16
HPC = 4          # heads per core
DK = 64
NCORES = 8

# ---------------------------------------------------------------------------
# Walrus in this toolchain allows only ONE semaphore wait per instruction and
# does not auto-split; stock TileContext emits multi-wait instructions (and a
# kernel-tail drain waiting on every proc). Split excess waits onto InstNoOp
# carriers inserted just before the offending instruction (same engine =>
# executes in order; sem-ge waits are monotonic so sequential == atomic).
# ---------------------------------------------------------------------------
from concourse.vector_clock import ScopedClock

_ctr = [0]


def _fix_excess_waits(nc):
    for f in nc.m.functions:
        for blk in f.blocks:
            insts = blk.instructions
            i = 0
            while i < len(insts):
                ins = insts[i]
                si = ins.sync_info
                n = len(si.on_wait) if si is not None else 0
                if n > 1:
                    waits = list(si.on_wait)
                    ins.sync_info = mybir.SyncInfo(
                        on_wait=waits[-1:], on_update=list(si.on_update)
                    )
                    for w in waits[:-1]:
                        nop = mybir.InstNoOp(
                            name=f"I-waitfix-{_ctr[0]}", ins=[], outs=[]
                        )
                        _ctr[0] += 1
                        nop.engine = ins.engine
                        nop.sync_info = mybir.SyncInfo(on_wait=[w], on_update=[])
                        try:
                            nc.register_instruction(nop, overwrite=True)
                        except Exception:
                            pass
                        insts.insert(i, nop)
                        i += 1
                i += 1


def _split_drain_and_barrier(self, tick_clock, wait_clock):
    nc = self.nc
    drain_inst = nc.sync.drain()
    wait_clock.add_sem_waits(
        drain_inst.ins, ScopedClock({None: tick_clock.global_clock})
    )
    _fix_excess_waits(nc)
    nc.all_engine_barrier()
    assert self.sems is not None
    popped = nc._tile_sem_poison_stack.pop()
    assert popped is self._sem_poison
    nc.clear_and_free_semaphores(list(self.sems.allocated().values()))
    nc.all_engine_barrier()


tile.TileContext._drain_and_barrier = _split_drain_and_barrier


# ---------------------------------------------------------------------------
# Device program (identical on all 8 cores; data differs per core)
# ---------------------------------------------------------------------------

def build_nc():
    nc = bass.Bass(trn_type="TRN2")

    qT = nc.dram_tensor("qT", [D, S], F32, kind="ExternalInput")
    kT = nc.dram_tensor("kT", [D, S], F32, kind="ExternalInput")
    vT = nc.dram_tensor("vT", [D, S], F32, kind="ExternalInput")
    wq = nc.dram_tensor("wq", [D, 256], F32, kind="ExternalInput")
    wk = nc.dram_tensor("wk", [D, 256], F32, kind="ExternalInput")
    wv = nc.dram_tensor("wv", [D, 256], F32, kind="ExternalInput")
    bq = nc.dram_tensor("bq", [256, 1], F32, kind="ExternalInput")
    bk = nc.dram_tensor("bk", [256, 1], F32, kind="ExternalInput")
    bv = nc.dram_tensor("bv", [1, 256], F32, kind="ExternalInput")
    wo = nc.dram_tensor("wo", [256, D], F32, kind="ExternalInput")
    bo = nc.dram_tensor("bo", [1, D], F32, kind="ExternalInput")
    mmT = nc.dram_tensor("mmT", [128, 16], F32, kind="ExternalInput")

    attnT = nc.dram_tensor("attnT", [HPC, S, S], F32, kind="ExternalOutput")
    partial = nc.dram_tensor("partial", [S, D], F32, kind="ExternalOutput")

    with tile.TileContext(nc) as tc:
        # --- persistent tiles (live for the whole kernel) ---
        pers_cm = tc.tile_pool(name="pers", bufs=1)
        pers = pers_cm.__enter__()
        QTe = pers.tile([128, 2, S], F32R)     # [pair-dims, pair, q]
        KTe = pers.tile([128, 2, S], F32R)     # [pair-dims, pair, k]
        V4 = pers.tile([128, 16, HPC * 65], F32R)  # [k-in-block, kblk, (h, dk+1)]
        combT = pers.tile([64, HPC, S], F32R)  # [dk, h, q] normalized heads out
        wo_sb = pers.tile([64, HPC, D], F32R)
        bo_sb = pers.tile([1, D], F32R)
        ones_sb = pers.tile([1, 128], F32R)
        ones32 = pers.tile([1, 128], F32)
        mm_sb = pers.tile([128, 16], F32)
        bq_sb = pers.tile([128, 2], F32)
        bk_sb = pers.tile([128, 2], F32)
        bv_sb = pers.tile([1, 256], F32R)

        nc.gpsimd.dma_start(wo_sb, wo.rearrange("(h p) n -> p h n", p=64))
        nc.gpsimd.dma_start(bo_sb, bo[:, :])
        nc.gpsimd.dma_start(bv_sb, bv[:, :])
        nc.sync.dma_start(mm_sb, mmT[:, :])
        nc.sync.dma_start(bq_sb, bq.rearrange("(a p) x -> p (a x)", p=128))
        nc.sync.dma_start(bk_sb, bk.rearrange("(a p) x -> p (a x)", p=128))
        nc.vector.memset(ones32, 1.0)
        nc.vector.tensor_copy(ones_sb, ones32)

        # --- phases P+V: projections ---
        with tc.tile_pool(name="pw", bufs=1) as pw:
            wq_sb = pw.tile([128, 8, 256], F32R, tag="wq")
            wk_sb = pw.tile([128, 8, 256], F32R, tag="wk")
            wv_sb = pw.tile([128, 8, 256], F32R, tag="wv")
            nc.gpsimd.dma_start(wq_sb, wq.rearrange("(c p) n -> p c n", p=128))
            nc.gpsimd.dma_start(wk_sb, wk.rearrange("(c p) n -> p c n", p=128))
            nc.gpsimd.dma_start(wv_sb, wv.rearrange("(c p) n -> p c n", p=128))

            # phase P: Q^T / K^T projections (heads packed in pairs)
            with tc.tile_pool(name="chunks", bufs=2) as chp, \
                 tc.tile_pool(name="qkps", bufs=2, space="PSUM") as qkp:
                for srcT, w_sb, b_sb, dstT in (
                    (qT, wq_sb, bq_sb, QTe),
                    (kT, wk_sb, bk_sb, KTe),
                ):
                    ps = [qkp.tile([128, S], F32, tag="qkps", name=f"qkps{i}") for i in range(2)]
                    for c in range(8):
                        ch = chp.tile([128, S], F32R, tag="ch")
                        nc.gpsimd.dma_start(ch, srcT[c * 128:(c + 1) * 128, :])
                        for pair in range(2):
                            for qc in range(4):
                                nc.tensor.matmul(
                                    ps[pair][:, qc * 512:(qc + 1) * 512],
                                    w_sb[:, c, pair * 128:(pair + 1) * 128],
                                    ch[:, qc * 512:(qc + 1) * 512],
                                    start=(c == 0),
                                    stop=(c == 7),
                                )
                    for pair in range(2):
                        nc.scalar.activation(
                            dstT[:, pair, :], ps[pair], AF.Identity,
                            bias=b_sb[:, pair:pair + 1],
                        )

            # phase V: V projection (4 heads packed, natural layout)
            with tc.tile_pool(name="vt", bufs=1) as vtp, \
                 tc.tile_pool(name="vps", bufs=2, space="PSUM") as vpp:
                vt = vtp.tile([128, 8, S], F32R)
                for c in range(8):
                    nc.gpsimd.dma_start(vt[:, c, :], vT[c * 128:(c + 1) * 128, :])
                V4r = V4.rearrange("p s (h x) -> p s h x", x=65)
                ones_col = vtp.tile([128, 16], F32, tag="ones_col")
                nc.vector.memset(ones_col, 1.0)
                for h in range(HPC):
                    nc.vector.tensor_copy(V4r[:, :, h, 64:65], ones_col.unsqueeze(2))
                for sb in range(16):
                    pv = vpp.tile([128, 256], F32, tag="vps")
                    for c in range(8):
                        nc.tensor.matmul(
                            pv, vt[:, c, sb * 128:(sb + 1) * 128], wv_sb[:, c, :],
                            start=(c == 0), stop=False,
                        )
                    nc.tensor.matmul(pv, ones_sb, bv_sb, start=False, stop=True)
                    nc.vector.tensor_copy(
                        V4r[:, sb, :, 0:64],
                        pv.rearrange("p (h x) -> p h x", x=64),
                    )
                # key-padding mask: zero masked V rows (incl. the ones column,
                # which makes Z = sum over unmasked keys only)
                for kb in range(16):
                    nc.vector.tensor_scalar_mul(
                        V4[:, kb, :], V4[:, kb, :], mm_sb[:, kb:kb + 1]
                    )

        # --- phase A: scores^T -> exp -> attn@V (+Z), normalize, write attn ---
        with tc.tile_pool(name="sc", bufs=2, space="PSUM") as scp, \
             tc.tile_pool(name="att", bufs=1, space="PSUM") as attp, \
             tc.tile_pool(name="rzbps", bufs=1, space="PSUM") as rzbpp, \
             tc.tile_pool(name="ex", bufs=18) as exp_pool, \
             tc.tile_pool(name="rzb", bufs=2) as rzbp, \
             tc.tile_pool(name="rz", bufs=4) as rzp:
            for h in range(HPC):
                pair, hr = h // 2, (h % 2) * 64
                for half in range(2):
                    att = attp.tile([65, 2, 512], F32, tag="att")
                    exs = []
                    for kb in range(16):
                        sc = scp.tile([128, 2, 512], F32, tag="sc")
                        for j in range(2):
                            qc = half * 2 + j
                            nc.tensor.matmul(
                                sc[:, j, :],
                                KTe[hr:hr + 64, pair, kb * 128:(kb + 1) * 128],
                                QTe[hr:hr + 64, pair, qc * 512:(qc + 1) * 512],
                                start=True, stop=True,
                            )
                        ex = exp_pool.tile([128, 2, 512], F32R, tag="ex")
                        nc.scalar.activation(ex, sc, AF.Exp)
                        for j in range(2):
                            nc.tensor.matmul(
                                att[:, j, :],
                                V4[:, kb, h * 65:(h + 1) * 65],
                                ex[:, j, :],
                                start=(kb == 0), stop=(kb == 15),
                            )
                        exs.append(ex)
                    # 1/Z, broadcast across partitions via ones^T @ rz matmul
                    rzb_ps = rzbpp.tile([128, 2, 512], F32, tag="rzbps")
                    for j in range(2):
                        rz = rzp.tile([1, 512], F32R, tag="rz")
                        with nc.allow_low_precision(reason="f32r rounding ok"):
                            nc.vector.reciprocal(rz, att[64:65, j, :])
                        nc.tensor.matmul(
                            rzb_ps[:, j, :], ones_sb, rz, start=True, stop=True
                        )
                    rzb = rzbp.tile([128, 2, 512], F32, tag="rzb")
                    nc.vector.tensor_copy(rzb, rzb_ps)
                    for j in range(2):
                        qc = half * 2 + j
                        nc.vector.tensor_mul(
                            combT[:, h, qc * 512:(qc + 1) * 512],
                            att[0:64, j, :],
                            rzb[0:64, j, :],
                        )
                    for kb in range(16):
                        nc.vector.scalar_tensor_tensor(
                            exs[kb], exs[kb], mm_sb[:, kb:kb + 1], rzb,
                            ALU.mult, ALU.mult,
                        )
                        nc.sync.dma_start(
                            attnT[h, kb * 128:(kb + 1) * 128,
                                  half * 1024:(half + 1) * 1024],
                            exs[kb].rearrange("p j x -> p (j x)").bitcast(F32),
                        )

        # --- phase O: out projection (row-parallel partial) ---
        with tc.tile_pool(name="ops", bufs=2, space="PSUM") as opp, \
             tc.tile_pool(name="ot", bufs=2) as otp:
            for sb in range(16):
                ot = otp.tile([128, 2, 512], F32, tag="ot")
                for nck in range(2):
                    po = opp.tile([128, 512], F32, tag="po")
                    for h in range(HPC):
                        nc.tensor.matmul(
                            po,
                            combT[:, h, sb * 128:(sb + 1) * 128],
                            wo_sb[:, h, nck * 512:(nck + 1) * 512],
                            start=(h == 0), stop=False,
                        )
                    nc.tensor.matmul(
                        po, ones_sb, bo_sb[:, nck * 512:(nck + 1) * 512],
                        start=False, stop=True,
                    )
                    nc.scalar.activation(ot[:, nck, :], po, AF.Copy)
                nc.sync.dma_start(
                    partial[sb * 128:(sb + 1) * 128, :],
                    ot.rearrange("p a x -> p (a x)"),
                )

        pers_cm.__exit__(None, None, None)
    return nc


_NC_CACHE = []


def _get_nc():
    if not _NC_CACHE:
        _NC_CACHE.append(build_nc())
    return _NC_CACHE[0]


# ---------------------------------------------------------------------------
# Host wrapper: shard, run SPMD on 8 cores, unshard
# ---------------------------------------------------------------------------

def kernel(query, key, value, mask, Wq, bq, Wk, bk, Wv, bv, Wo, bo):
    from concourse.bass_utils import run_bass_kernel_spmd

    query = np.asarray(query, dtype=np.float32)
    key = np.asarray(key, dtype=np.float32)
    value = np.asarray(value, dtype=np.float32)
    mask = np.asarray(mask)
    Wq = np.asarray(Wq, dtype=np.float32)
    bq = np.asarray(bq, dtype=np.float32)
    Wk = np.asarray(Wk, dtype=np.float32)
    bk = np.asarray(bk, dtype=np.float32)
    Wv = np.asarray(Wv, dtype=np.float32)
    bv = np.asarray(bv, dtype=np.float32)
    Wo = np.asarray(Wo, dtype=np.float32)
    bo = np.asarray(bo, dtype=np.float32)

    scale = 1.0 / np.sqrt(np.float32(DK))
    qTs = [np.ascontiguousarray(query[b].T) for b in range(B)]
    kTs = [np.ascontiguousarray(key[b].T) for b in range(B)]
    vTs = [np.ascontiguousarray(value[b].T) for b in range(B)]
    mmTs = [
        np.ascontiguousarray(mask[b].astype(np.float32).reshape(16, 128).T)
        for b in range(B)
    ]

    in_maps = []
    for c in range(NCORES):
        b, hg = c // HPC, c % HPC
        sl = slice(hg * 256, (hg + 1) * 256)
        in_maps.append({
            "qT": qTs[b],
            "kT": kTs[b],
            "vT": vTs[b],
            "wq": np.ascontiguousarray(Wq[:, sl]) * scale,
            "wk": np.ascontiguousarray(Wk[:, sl]),
            "wv": np.ascontiguousarray(Wv[:, sl]),
            "bq": (bq[sl] * scale).reshape(256, 1).copy(),
            "bk": bk[sl].reshape(256, 1).copy(),
            "bv": bv[sl].reshape(1, 256).copy(),
            "wo": np.ascontiguousarray(Wo[sl, :]),
            "bo": (bo if hg == 0 else np.zeros_like(bo)).reshape(1, D).copy(),
            "mmT": mmTs[b],
        })

    nc = _get_nc()
    res = run_bass_kernel_spmd(nc, in_maps, core_ids=list(range(NCORES)))

    # output = sum of the 4 row-parallel partials per batch (bo included once)
    out = np.stack([
        sum(res.results[b * HPC + hg]["partial"] for hg in range(HPC))
        for b in range(B)
    ]).astype(np.float32)

    # attn: device wrote [hg-local head, k, q]; return a transposed view
    stacked = np.stack([res.results[c]["attnT"] for c in range(NCORES)])
    attn = stacked.reshape(B, H, S, S).transpose(0, 1, 3, 2)
    return out, attn


# revision 15
# speedup vs baseline: 605.6071x; 605.6071x over previous
"""Multi-head attention (B=2, S=2048, D=1024, H=16) on 8 TRN2 NeuronCores.

Sharding: core c handles batch b = c//4 and head-group hg = c%4 (4 heads).
Device computes, per core:
  Q^T,K^T = (Wq_hg^T q^T + bq), (Wk_hg^T k^T + bk)    [dk-major layouts]
  V4      = v @ Wv_hg + bv, rows masked by key-padding mask, plus a ones
            column per head (yields softmax denominators Z for free)
  scoresT = K @ Q^T per head (scaled via Wq pre-scaling), exp on ScalarE,
  outT    = V_ext^T @ exp  (row 64 = Z), attn = exp * mask * (1/Z)
  partial = (outT/Z heads combined) @ Wo_hg  (+ bo on hg==0 cores)
Host: pre-transposes q/k/v per batch, slices weights per head-group, sums the
4 row-parallel partials per batch, and reassembles attn (device writes attn
transposed [k,q]; the return is a zero-copy transposed view).

All matmuls run in float32r (TF32-like, ~1e-4 rel err, full PE rate).
"""

import sys

sys.path.insert(0, "/opt/trn_rl_repo")

import numpy as np

import concourse.bass as bass
import concourse.mybir as mybir
import concourse.tile as tile

F32 = mybir.dt.float32
F32R = mybir.dt.float32r
AF = mybir.ActivationFunctionType
ALU = mybir.AluOpType

B, S, D, H = 2, 2048, 1024, 16
HPC = 4          # heads per core
DK = 64
NCORES = 8

# ---------------------------------------------------------------------------
# Walrus in this toolchain allows only ONE semaphore wait per instruction and
# does not auto-split; stock TileContext emits multi-wait instructions (and a
# kernel-tail drain waiting on every proc). Split excess waits onto InstNoOp
# carriers inserted just before the offending instruction (same engine =>
# executes in order; sem-ge waits are monotonic so sequential == atomic).
# ---------------------------------------------------------------------------
from concourse.vector_clock import ScopedClock

_ctr = [0]


def _fix_excess_waits(nc):
    for f in nc.m.functions:
        for blk in f.blocks:
            insts = blk.instructions
            i = 0
            while i < len(insts):
                ins = insts[i]
                si = ins.sync_info
                n = len(si.on_wait) if si is not None else 0
                if n > 1:
                    waits = list(si.on_wait)
                    ins.sync_info = mybir.SyncInfo(
                        on_wait=waits[-1:], on_update=list(si.on_update)
                    )
                    for w in waits[:-1]:
                        nop = mybir.InstNoOp(
                            name=f"I-waitfix-{_ctr[0]}", ins=[], outs=[]
                        )
                        _ctr[0] += 1
                        nop.engine = ins.engine
                        nop.sync_info = mybir.SyncInfo(on_wait=[w], on_update=[])
                        try:
                            nc.register_instruction(nop, overwrite=True)
                        except Exception:
                            pass
                        insts.insert(i, nop)
                        i += 1
                i += 1


def _split_drain_and_barrier(self, tick_clock, wait_clock):
    nc = self.nc
    drain_inst = nc.sync.drain()
    wait_clock.add_sem_waits(
        drain_inst.ins, ScopedClock({None: tick_clock.global_clock})
    )
    _fix_excess_waits(nc)
    nc.all_engine_barrier()
    assert self.sems is not None
    popped = nc._tile_sem_poison_stack.pop()
    assert popped is self._sem_poison
    nc.clear_and_free_semaphores(list(self.sems.allocated().values()))
    nc.all_engine_barrier()


tile.TileContext._drain_and_barrier = _split_drain_and_barrier


# ---------------------------------------------------------------------------
# Device program (identical on all 8 cores; data differs per core)
# ---------------------------------------------------------------------------

def build_nc():
    nc = bass.Bass(trn_type="TRN2", enable_partition_id=False)

    qT = nc.dram_tensor("qT", [D, S], F32, kind="ExternalInput")
    kT = nc.dram_tensor("kT", [D, S], F32, kind="ExternalInput")
    vT = nc.dram_tensor("vT", [D, S], F32, kind="ExternalInput")
    wq = nc.dram_tensor("wq", [D, 256], F32, kind="ExternalInput")
    wk = nc.dram_tensor("wk", [D, 256], F32, kind="ExternalInput")
    wv = nc.dram_tensor("wv", [D, 256], F32, kind="ExternalInput")
    bq = nc.dram_tensor("bq", [256, 1], F32, kind="ExternalInput")
    bk = nc.dram_tensor("bk", [256, 1], F32, kind="ExternalInput")
    bv = nc.dram_tensor("bv", [1, 256], F32, kind="ExternalInput")
    wo = nc.dram_tensor("wo", [256, D], F32, kind="ExternalInput")
    bo = nc.dram_tensor("bo", [1, D], F32, kind="ExternalInput")
    mmT = nc.dram_tensor("mmT", [128, 16], F32, kind="ExternalInput")

    attnT = nc.dram_tensor("attnT", [HPC, S, S], F32, kind="ExternalOutput")
    partial = nc.dram_tensor("partial", [S, D], F32, kind="ExternalOutput")

    with tile.TileContext(nc) as tc:
        # --- persistent tiles (live for the whole kernel) ---
        pers_cm = tc.tile_pool(name="pers", bufs=1)
        pers = pers_cm.__enter__()
        QTe = pers.tile([128, 2, S], F32R)     # [pair-dims, pair, q]
        KTe = pers.tile([128, 2, S], F32R)     # [pair-dims, pair, k]
        V4 = pers.tile([128, 16, HPC * 65], F32R)  # [k-in-block, kblk, (h, dk+1)]
        combT = pers.tile([64, HPC, S], F32R)  # [dk, h, q] normalized heads out
        wo_sb = pers.tile([64, HPC, D], F32R)
        bo_sb = pers.tile([1, D], F32R)
        ones_sb = pers.tile([1, 128], F32R)
        ones32 = pers.tile([1, 128], F32)
        mm_sb = pers.tile([128, 16], F32)
        bq_sb = pers.tile([128, 2], F32)
        bk_sb = pers.tile([128, 2], F32)
        bv_sb = pers.tile([1, 256], F32R)

        nc.gpsimd.dma_start(wo_sb, wo.rearrange("(h p) n -> p h n", p=64))
        nc.gpsimd.dma_start(bo_sb, bo[:, :])
        nc.gpsimd.dma_start(bv_sb, bv[:, :])
        nc.sync.dma_start(mm_sb, mmT[:, :])
        nc.sync.dma_start(bq_sb, bq.rearrange("(a p) x -> p (a x)", p=128))
        nc.sync.dma_start(bk_sb, bk.rearrange("(a p) x -> p (a x)", p=128))
        nc.vector.memset(ones32, 1.0)
        nc.vector.tensor_copy(ones_sb, ones32)

        # --- phases P+V: projections ---
        with tc.tile_pool(name="pw", bufs=1) as pw:
            wq_sb = pw.tile([128, 8, 256], F32R, tag="wq")
            wk_sb = pw.tile([128, 8, 256], F32R, tag="wk")
            wv_sb = pw.tile([128, 8, 256], F32R, tag="wv")
            nc.gpsimd.dma_start(wq_sb, wq.rearrange("(c p) n -> p c n", p=128))
            nc.gpsimd.dma_start(wk_sb, wk.rearrange("(c p) n -> p c n", p=128))
            nc.gpsimd.dma_start(wv_sb, wv.rearrange("(c p) n -> p c n", p=128))

            # phase P: Q^T / K^T projections (heads packed in pairs)
            with tc.tile_pool(name="chunks", bufs=2) as chp, \
                 tc.tile_pool(name="qkps", bufs=2, space="PSUM") as qkp:
                for srcT, w_sb, b_sb, dstT in (
                    (qT, wq_sb, bq_sb, QTe),
                    (kT, wk_sb, bk_sb, KTe),
                ):
                    ps = [qkp.tile([128, S], F32, tag="qkps", name=f"qkps{i}") for i in range(2)]
                    for c in range(8):
                        ch = chp.tile([128, S], F32R, tag="ch")
                        nc.gpsimd.dma_start(ch, srcT[c * 128:(c + 1) * 128, :])
                        for pair in range(2):
                            for qc in range(4):
                                nc.tensor.matmul(
                                    ps[pair][:, qc * 512:(qc + 1) * 512],
                                    w_sb[:, c, pair * 128:(pair + 1) * 128],
                                    ch[:, qc * 512:(qc + 1) * 512],
                                    start=(c == 0),
                                    stop=(c == 7),
                                )
                    for pair in range(2):
                        nc.scalar.activation(
                            dstT[:, pair, :], ps[pair], AF.Identity,
                            bias=b_sb[:, pair:pair + 1],
                        )

            # phase V: V projection (4 heads packed, natural layout)
            with tc.tile_pool(name="vt", bufs=1) as vtp, \
                 tc.tile_pool(name="vps", bufs=2, space="PSUM") as vpp:
                vt = vtp.tile([128, 8, S], F32R)
                for c in range(8):
                    nc.gpsimd.dma_start(vt[:, c, :], vT[c * 128:(c + 1) * 128, :])
                V4r = V4.rearrange("p s (h x) -> p s h x", x=65)
                ones_col = vtp.tile([128, 16], F32, tag="ones_col")
                nc.vector.memset(ones_col, 1.0)
                for h in range(HPC):
                    nc.vector.tensor_copy(V4r[:, :, h, 64:65], ones_col.unsqueeze(2))
                for sb in range(16):
                    pv = vpp.tile([128, 256], F32, tag="vps")
                    for c in range(8):
                        nc.tensor.matmul(
                            pv, vt[:, c, sb * 128:(sb + 1) * 128], wv_sb[:, c, :],
                            start=(c == 0), stop=False,
                        )
                    nc.tensor.matmul(pv, ones_sb, bv_sb, start=False, stop=True)
                    nc.vector.tensor_copy(
                        V4r[:, sb, :, 0:64],
                        pv.rearrange("p (h x) -> p h x", x=64),
                    )
                # key-padding mask: zero masked V rows (incl. the ones column,
                # which makes Z = sum over unmasked keys only)
                for kb in range(16):
                    nc.vector.tensor_scalar_mul(
                        V4[:, kb, :], V4[:, kb, :], mm_sb[:, kb:kb + 1]
                    )

        # --- phase A: scores^T -> exp -> attn@V (+Z), normalize, write attn ---
        with tc.tile_pool(name="sc", bufs=2, space="PSUM") as scp, \
             tc.tile_pool(name="att", bufs=1, space="PSUM") as attp, \
             tc.tile_pool(name="rzbps", bufs=1, space="PSUM") as rzbpp, \
             tc.tile_pool(name="ex", bufs=18) as exp_pool, \
             tc.tile_pool(name="rzb", bufs=2) as rzbp, \
             tc.tile_pool(name="rz", bufs=4) as rzp:
            for h in range(HPC):
                pair, hr = h // 2, (h % 2) * 64
                for half in range(2):
                    att = attp.tile([65, 2, 512], F32, tag="att")
                    exs = []
                    for kb in range(16):
                        sc = scp.tile([128, 2, 512], F32, tag="sc")
                        for j in range(2):
                            qc = half * 2 + j
                            nc.tensor.matmul(
                                sc[:, j, :],
                                KTe[hr:hr + 64, pair, kb * 128:(kb + 1) * 128],
                                QTe[hr:hr + 64, pair, qc * 512:(qc + 1) * 512],
                                start=True, stop=True,
                            )
                        ex = exp_pool.tile([128, 2, 512], F32R, tag="ex")
                        nc.scalar.activation(ex, sc, AF.Exp)
                        for j in range(2):
                            nc.tensor.matmul(
                                att[:, j, :],
                                V4[:, kb, h * 65:(h + 1) * 65],
                                ex[:, j, :],
                                start=(kb == 0), stop=(kb == 15),
                            )
                        exs.append(ex)
                    # 1/Z, broadcast across partitions via ones^T @ rz matmul
                    rzb_ps = rzbpp.tile([128, 2, 512], F32, tag="rzbps")
                    for j in range(2):
                        rz = rzp.tile([1, 512], F32R, tag="rz")
                        with nc.allow_low_precision(reason="f32r rounding ok"):
                            nc.vector.reciprocal(rz, att[64:65, j, :])
                        nc.tensor.matmul(
                            rzb_ps[:, j, :], ones_sb, rz, start=True, stop=True
                        )
                    rzb = rzbp.tile([128, 2, 512], F32, tag="rzb")
                    nc.vector.tensor_copy(rzb, rzb_ps)
                    for j in range(2):
                        qc = half * 2 + j
                        nc.vector.tensor_mul(
                            combT[:, h, qc * 512:(qc + 1) * 512],
                            att[0:64, j, :],
                            rzb[0:64, j, :],
                        )
                    for kb in range(16):
                        nc.vector.scalar_tensor_tensor(
                            exs[kb], exs[kb], mm_sb[:, kb:kb + 1], rzb,
                            ALU.mult, ALU.mult,
                        )
                        nc.sync.dma_start(
                            attnT[h, kb * 128:(kb + 1) * 128,
                                  half * 1024:(half + 1) * 1024],
                            exs[kb].rearrange("p j x -> p (j x)").bitcast(F32),
                        )

        # --- phase O: out projection (row-parallel partial) ---
        with tc.tile_pool(name="ops", bufs=2, space="PSUM") as opp, \
             tc.tile_pool(name="ot", bufs=2) as otp:
            for sb in range(16):
                ot = otp.tile([128, 2, 512], F32, tag="ot")
                for nck in range(2):
                    po = opp.tile([128, 512], F32, tag="po")
                    for h in range(HPC):
                        nc.tensor.matmul(
                            po,
                            combT[:, h, sb * 128:(sb + 1) * 128],
                            wo_sb[:, h, nck * 512:(nck + 1) * 512],
                            start=(h == 0), stop=False,
                        )
                    nc.tensor.matmul(
                        po, ones_sb, bo_sb[:, nck * 512:(nck + 1) * 512],
                        start=False, stop=True,
                    )
                    nc.scalar.activation(ot[:, nck, :], po, AF.Copy)
                nc.sync.dma_start(
                    partial[sb * 128:(sb + 1) * 128, :],
                    ot.rearrange("p a x -> p (a x)"),
                )

        pers_cm.__exit__(None, None, None)
    return nc


_NC_CACHE = []


def _get_nc():
    if not _NC_CACHE:
        _NC_CACHE.append(build_nc())
    return _NC_CACHE[0]


# ---------------------------------------------------------------------------
# Cached PJRT executor: trace/compile once, reuse across kernel() calls.
# Mirrors bass2jax.run_bass_via_pjrt's multi-core path (no partition-id /
# debug tensors in this program).
# ---------------------------------------------------------------------------
_EXEC_CACHE = []


def _make_executor():
    import jax
    from jax.experimental.shard_map import shard_map
    from jax.sharding import Mesh, PartitionSpec
    from concourse import bass2jax

    bass2jax.install_neuronx_cc_hook()
    nc = _get_nc()
    assert nc.partition_id_tensor is None and nc.dbg_addr is None

    in_names, out_names, out_avals = [], [], []
    for alloc in nc.m.functions[0].allocations:
        if not isinstance(alloc, mybir.MemoryLocationSet):
            continue
        name = alloc.memorylocations[0].name
        if alloc.kind == "ExternalInput":
            in_names.append(name)
        elif alloc.kind == "ExternalOutput":
            out_names.append(name)
            out_avals.append(
                jax.core.ShapedArray(
                    tuple(alloc.tensor_shape), mybir.dt.np(alloc.dtype)
                )
            )
    n_params = len(in_names)
    all_in_names = tuple(in_names + out_names)
    donate = tuple(range(n_params, n_params + len(out_names)))

    def _body(*args):
        outs = bass2jax._bass_exec_p.bind(
            *args,
            out_avals=tuple(out_avals),
            in_names=all_in_names,
            out_names=tuple(out_names),
            lowering_input_output_aliases=(),
            sim_require_finite=True,
            sim_require_nnan=True,
            nc=nc,
        )
        return tuple(outs)

    devices = jax.devices()[:NCORES]
    mesh = Mesh(np.asarray(devices), ("core",))
    spec = PartitionSpec("core")
    sharded = jax.jit(
        shard_map(
            _body,
            mesh=mesh,
            in_specs=(spec,) * (n_params + len(out_names)),
            out_specs=(spec,) * len(out_names),
            check_rep=False,
        ),
        donate_argnums=donate,
        keep_unused=True,
    )

    zero_shapes = [
        ((NCORES * av.shape[0],) + tuple(av.shape[1:]), av.dtype)
        for av in out_avals
    ]
    make_zeros = jax.jit(
        lambda: tuple(
            jax.numpy.zeros(shp, dt) for shp, dt in zero_shapes
        ),
        out_shardings=tuple(
            jax.sharding.NamedSharding(mesh, spec) for _ in zero_shapes
        ),
    )

    return {
        "sharded": sharded,
        "make_zeros": make_zeros,
        "in_names": in_names,
        "out_names": out_names,
        "out_avals": out_avals,
        "mesh": mesh,
        "spec": spec,
    }


def _get_executor():
    if not _EXEC_CACHE:
        _EXEC_CACHE.append(_make_executor())
    return _EXEC_CACHE[0]


def _run_spmd(in_maps):
    """Run the program on 8 cores; returns list of per-core output dicts."""
    import jax
    from jax.sharding import NamedSharding

    ex = _get_executor()
    concat_in = [
        np.concatenate([m[name] for m in in_maps], axis=0)
        for name in ex["in_names"]
    ]
    sharding = NamedSharding(ex["mesh"], ex["spec"])
    dev_in = [jax.device_put(a, sharding) for a in concat_in]
    zeros = ex["make_zeros"]()
    out_arrs = ex["sharded"](*dev_in, *zeros)
    outs = {
        name: np.asarray(out_arrs[i]).reshape(
            NCORES, *ex["out_avals"][i].shape
        )
        for i, name in enumerate(ex["out_names"])
    }
    return [{name: outs[name][c] for name in outs} for c in range(NCORES)]


# ---------------------------------------------------------------------------
# Host wrapper: shard, run SPMD on 8 cores, unshard
# ---------------------------------------------------------------------------

def kernel(query, key, value, mask, Wq, bq, Wk, bk, Wv, bv, Wo, bo):
    query = np.asarray(query, dtype=np.float32)
    key = np.asarray(key, dtype=np.float32)
    value = np.asarray(value, dtype=np.float32)
    mask = np.asarray(mask)
    Wq = np.asarray(Wq, dtype=np.float32)
    bq = np.asarray(bq, dtype=np.float32)
    Wk = np.asarray(Wk, dtype=np.float32)
    bk = np.asarray(bk, dtype=np.float32)
    Wv = np.asarray(Wv, dtype=np.float32)
    bv = np.asarray(bv, dtype=np.float32)
    Wo = np.asarray(Wo, dtype=np.float32)
    bo = np.asarray(bo, dtype=np.float32)

    scale = 1.0 / np.sqrt(np.float32(DK))
    qTs = [np.ascontiguousarray(query[b].T) for b in range(B)]
    kTs = [np.ascontiguousarray(key[b].T) for b in range(B)]
    vTs = [np.ascontiguousarray(value[b].T) for b in range(B)]
    mmTs = [
        np.ascontiguousarray(mask[b].astype(np.float32).reshape(16, 128).T)
        for b in range(B)
    ]

    in_maps = []
    for c in range(NCORES):
        b, hg = c // HPC, c % HPC
        sl = slice(hg * 256, (hg + 1) * 256)
        in_maps.append({
            "qT": qTs[b],
            "kT": kTs[b],
            "vT": vTs[b],
            "wq": np.ascontiguousarray(Wq[:, sl]) * scale,
            "wk": np.ascontiguousarray(Wk[:, sl]),
            "wv": np.ascontiguousarray(Wv[:, sl]),
            "bq": (bq[sl] * scale).reshape(256, 1).copy(),
            "bk": bk[sl].reshape(256, 1).copy(),
            "bv": bv[sl].reshape(1, 256).copy(),
            "wo": np.ascontiguousarray(Wo[sl, :]),
            "bo": (bo if hg == 0 else np.zeros_like(bo)).reshape(1, D).copy(),
            "mmT": mmTs[b],
        })

    results = _run_spmd(in_maps)

    # output = sum of the 4 row-parallel partials per batch (bo included once)
    out = np.stack([
        sum(results[b * HPC + hg]["partial"] for hg in range(HPC))
        for b in range(B)
    ]).astype(np.float32)

    # attn: device wrote [hg-local head, k, q]; return a transposed view
    stacked = np.stack([results[c]["attnT"] for c in range(NCORES)])
    attn = stacked.reshape(B, H, S, S).transpose(0, 1, 3, 2)
    return out, attn
